# revision 1
# baseline (speedup 1.0000x reference)
"""DDSP core synthesizer kernel for Trainium2 (8 NeuronCores, data-parallel).

Reference computation (per row of B=32, T=64000):
    f0_hz = 20*exp(f0); phase = cumsum(2*pi*f0_hz/SR)
    hw    = sum_k sin(phase*k)/k   (k = 1..60)
    audio = mix*hw*loud + (1-mix)*noise*loud;  out = audio / (max|audio| + 1e-6)

Device algorithm (everything in "turns" = phase/2pi):
    inc  = exp(f0 + ln(20/SR))                       [ACT Exp]
    u    = blocked cumsum of inc                      [DVE scan + PE triangular mm]
    u1   = u - rint(u)                                [DVE magic-number rint]
    u_hi = f16(u1); u_lo = f16((u1-u_hi)*4096)        [exact 2-term split]
    per harmonic block (60 k's x 2 time-blocks on 120 partitions):
        x  = k*u_hi + (k/4096)*u_lo                   [PE f16 matmul, exact]
        r  = rint(x)                                  [DVE dual-op tensor_scalar]
        v  = x - r   in [-0.5, 0.5]                   [PE -identity @ r accumulate]
        s  = sin(2*pi*v)                              [ACT Sin, scale=2pi]
        hw += (1/k)^T @ s                             [PE fp32r matmul]
    epilogue: audio = loud*(noise + mix*(hw - noise)); peak-normalize per row
              (row max via free-reduce + DVE 32x32 transpose trick).

Sharding: pure data parallel, 4 rows per core, SPMD on cores 0-7.
"""

import sys

sys.path.insert(0, "/opt/trn_rl_repo")

import numpy as np
import ml_dtypes
from contextlib import ExitStack

import concourse.bass as bass
import concourse.tile as tile
from concourse import bacc, mybir
from concourse import bass_utils

f32 = np.float32
dt = mybir.dt

SR = 44100.0
H = 60                      # harmonics
B, T = 32, 64000
NCORES = 8
RPC = B // NCORES           # rows per core = 4
P = 128                     # SBUF partitions
FD = T * RPC // P           # free dim of master tiles = 2000
BPR = P // RPC              # blocks per row = 32
NPAIR = P // 2              # block pairs = 64
PI = float(np.pi)
MAGIC = float(1.5 * 2.0 ** 23)
LO_SCALE = 4096.0
Q_OFFS = [0, 512, 1024, 1536]
Q_LENS = [512, 512, 512, 464]
EXP_BIAS = float(np.log(20.0 / SR))

_cache = {}


def _consts():
    # lt: exclusive-prefix matmul weights. offs[m] = sum_k lt[k, m] * totals[k]
    kk, mm_ = np.meshgrid(np.arange(P), np.arange(P), indexing="ij")
    lt = ((kk // BPR == mm_ // BPR) & (kk % BPR < mm_ % BPR)).astype(f32)

    # Stage-2 partitioning: per pass, 64 local blocks x 2 harmonics fill
    # 128 partitions (p = 2*b_loc + kap, k = 2*kg + kap + 1; kg = 0..29).
    # Group tile uhalf[g] holds local block b at partitions 2b (hi), 2b+1 (lo).
    # xsel[kg]: lhsT [128, 128], x[2b+kap] = k*u_hi[b] + (k/4096)*u_lo[b].
    # wsel[kg]: lhsT [128, 64], hw[b] += sum_kap (1/k) * s[2b+kap].
    xsel = np.zeros((30, P, P), dtype=np.float64)
    wsel = np.zeros((30, P, 64), dtype=f32)  # cast at return
    negi = np.zeros((P, P), dtype=np.float64)
    for p in range(P):
        negi[p, p] = -1.0
    for kg in range(30):
        for b in range(64):
            for kap in range(2):
                k = 2 * kg + kap + 1.0
                xsel[kg, 2 * b + 0, 2 * b + kap] = k
                xsel[kg, 2 * b + 1, 2 * b + kap] = k / LO_SCALE
                wsel[kg, 2 * b + kap, b] = 1.0 / k
    xsel = xsel.astype(np.float16)
    negi = negi.astype(np.float16)
    wsel = wsel.astype(ml_dtypes.bfloat16)
    return {"lt": lt, "xsel": xsel, "wsel": wsel, "negi": negi}


def _build(xchunk=512, xbufs=6, sbufs=6, hsec=4, reps=1, act_pct=0, pool_epi=True, hbufs=2):
    nc = bacc.Bacc("TRN2", target_bir_lowering=False, debug=False,
                   enable_asserts=True, num_devices=NCORES)

    f0_d = nc.dram_tensor("f0", [P, FD], dt.float32, kind="ExternalInput")
    loud_d = nc.dram_tensor("loud", [P, FD], dt.float32, kind="ExternalInput")
    mix_d = nc.dram_tensor("mix", [P, FD], dt.float32, kind="ExternalInput")
    noise_d = nc.dram_tensor("noise", [P, FD], dt.float32, kind="ExternalInput")
    lt_d = nc.dram_tensor("lt", [P, P], dt.float32, kind="ExternalInput")
    xsel_d = nc.dram_tensor("xsel", [30, P, P], dt.float16, kind="ExternalInput")
    wsel_d = nc.dram_tensor("wsel", [30, P, 64], dt.bfloat16, kind="ExternalInput")
    negi_d = nc.dram_tensor("negi", [P, P], dt.float16, kind="ExternalInput")
    out_d = nc.dram_tensor("audio", [P, FD], dt.float32, kind="ExternalOutput")

    AF = mybir.ActivationFunctionType
    ALU = mybir.AluOpType

    with tile.TileContext(nc) as tc, ExitStack() as ctx:
        pool = ctx.enter_context(tc.tile_pool(name="sb", bufs=1))
        spool = ctx.enter_context(tc.tile_pool(name="sin", bufs=sbufs))
        rpool = ctx.enter_context(tc.tile_pool(name="rint", bufs=sbufs))
        xpool = ctx.enter_context(tc.tile_pool(name="xps", bufs=xbufs, space="PSUM"))
        hpool = ctx.enter_context(tc.tile_pool(name="hps", bufs=hbufs, space="PSUM"))

        def const_col(val, tag):
            t = pool.tile([P, 1], dt.float32, tag=tag)
            nc.vector.memset(t[:], val)
            return t

        exp_bias = const_col(EXP_BIAS, "cbias_exp")
        zero_bias = const_col(0.0, "cbias_zero")
        mag_bias = const_col(MAGIC, "cbias_mag")
        nmag_bias = const_col(-MAGIC, "cbias_nmag")

        # ---- input DMA ----
        f0 = pool.tile([P, FD], dt.float32, tag="scr", bufs=4, name="f0")
        nc.sync.dma_start(f0[:], f0_d.ap())
        lt = pool.tile([P, P], dt.float32)
        nc.gpsimd.dma_start(lt[:], lt_d.ap())
        xsel = pool.tile([P, 30, P], dt.float16)
        nc.gpsimd.dma_start(xsel[:], xsel_d.ap().rearrange("g p m -> p g m"))
        wsel = pool.tile([P, 30, 64], dt.bfloat16)
        nc.gpsimd.dma_start(wsel[:], wsel_d.ap().rearrange("g p m -> p g m"))
        negi = pool.tile([P, P], dt.float16)
        nc.gpsimd.dma_start(negi[:], negi_d.ap())
        # ---- stage 1: phase accumulation (turns) ----
        inc = pool.tile([P, FD], dt.float32, tag="scr", bufs=4, name="inc")
        nc.scalar.activation(inc[:], f0[:], AF.Exp, bias=exp_bias[:, 0:1], scale=1.0)

        local = pool.tile([P, FD], dt.float32, tag="scr", bufs=4, name="local")
        nc.vector.tensor_tensor_scan(local[:], inc[:], inc[:], 0.0,
                                     ALU.add, ALU.bypass)

        offs_ps = xpool.tile([P, 1], dt.float32, tag="x")
        nc.tensor.matmul(offs_ps[:], lt[:], local[:, FD - 1:FD],
                         start=True, stop=True)
        offs = pool.tile([P, 1], dt.float32)
        nc.vector.tensor_copy(offs[:], offs_ps[:])

        u = pool.tile([P, FD], dt.float32, tag="scr", bufs=4, name="u")
        nc.vector.tensor_scalar(u[:], local[:], offs[:, 0:1], None, ALU.add)
        # u1 = u - rint(u)  (safe: k*(u - n) == k*u mod 1)
        ur = pool.tile([P, FD], dt.float32, tag="scr", bufs=4, name="ur")
        nc.vector.tensor_scalar(ur[:], u[:], MAGIC, MAGIC, ALU.add, ALU.subtract)
        u1 = pool.tile([P, FD], dt.float32, tag="scr", bufs=4, name="u1")
        nc.gpsimd.tensor_tensor(u1[:], u[:], ur[:], ALU.subtract)

        # ---- f16 hi/lo split ----
        uhi = pool.tile([P, FD], dt.float16)
        nc.vector.tensor_copy(uhi[:], u1[:])
        ulo_f32 = pool.tile([P, FD], dt.float32, tag="scr", bufs=4, name="ulo_f32")
        nc.gpsimd.tensor_tensor(ulo_f32[:], u1[:], uhi[:], ALU.subtract)
        ulo = pool.tile([P, FD], dt.float16)
        nc.vector.tensor_scalar(ulo[:], ulo_f32[:], LO_SCALE, None, ALU.mult)

        # ---- repack: group tile g holds local block b=0..63 (global 64g+b)
        # at partitions 2b (hi) and 2b+1 (lo) ----
        uhi_v = uhi[:].rearrange("(g b) f -> g b f", g=2)
        ulo_v = ulo[:].rearrange("(g b) f -> g b f", g=2)
        uhalf = []
        for g in range(2):
            t = pool.tile([P, FD], dt.float16, tag=f"uhl{g}")
            tv = t[:].rearrange("(b s) f -> b s f", s=2)
            nc.sync.dma_start(tv[:, 0, :], uhi_v[g])
            nc.scalar.dma_start(tv[:, 1, :], ulo_v[g])
            uhalf.append(t)

        # epilogue inputs (issued after phase DMAs so they don't compete
        # for HBM bandwidth on the critical path)
        louds, mixs, noises = [], [], []
        for g in range(2):
            lg = pool.tile([64, FD], dt.float32, tag=f"loud{g}")
            nc.scalar.dma_start(lg[:], loud_d.ap()[64 * g:64 * g + 64, :])
            louds.append(lg)
            mg = pool.tile([64, FD], dt.float32, tag=f"mix{g}")
            nc.scalar.dma_start(mg[:], mix_d.ap()[64 * g:64 * g + 64, :])
            mixs.append(mg)
            ng = pool.tile([64, FD], dt.float32, tag=f"noise{g}")
            nc.scalar.dma_start(ng[:], noise_d.ap()[64 * g:64 * g + 64, :])
            noises.append(ng)

        # ---- stage 2 + 3: harmonic bank and epilogue, per block-group ----
        rep_ctx = tc.For_i(0, reps, 1) if reps > 1 else None
        if rep_ctx is not None:
            rep_ctx.__enter__()
        # hw_g[b_loc, t] (b_loc = 0..63, global block 64g + b_loc) accumulates
        # over 30 kg-passes at PSUM partitions 0..63 (col-group 3 of the PE
        # array cannot be a tiled matmul destination, so both groups write
        # at base 0 and the epilogue runs per group). The free dim is
        # processed in `hsec` sections to free PSUM banks for x buffers.
        pass_ctr = [0]
        sec_bounds = []
        spos = 0
        nbank = 4 // hsec
        for si in range(hsec):
            slen = min(nbank * 512, FD - spos)
            sec_bounds.append((spos, slen))
            spos += slen
        audio0 = pool.tile([64, FD], dt.float32, tag="audio0", name="audio0")
        audio1 = pool.tile([64, FD], dt.float32, tag="audio1", name="audio1")
        audios = [audio0, audio1]
        for (s0, sn_) in sec_bounds:
            for g in range(2):
                audio = audios[g]
                hw_g = hpool.tile([64, nbank, 512], dt.float32, tag="hw")
                chunks = []
                c0 = s0
                while c0 < s0 + sn_:
                    cn = min(xchunk, s0 + sn_ - c0)
                    chunks.append((c0, cn))
                    c0 += cn
                for (c0, cn) in chunks:
                    for kg in range(30):
                        x_ps = xpool.tile([P, xchunk], dt.float32, tag="x")
                        # x = k*u_hi + (k/4096)*u_lo  (exact f16 products)
                        for qo in range(0, cn, 512):
                            qn = min(512, cn - qo)
                            nc.tensor.matmul(
                                x_ps[:, qo:qo + qn],
                                xsel[:, kg, :],
                                uhalf[g][:, c0 + qo:c0 + qo + qn],
                                start=True, stop=False)
                        r = rpool.tile([P, xchunk], dt.float16, tag="r")
                        pass_ctr[0] += 1
                        if (pass_ctr[0] * act_pct) // 100 != ((pass_ctr[0] - 1) * act_pct) // 100:
                            # rint on ACT: t = Id(x + M); r = Id(t - M)
                            tti = spool.tile([P, xchunk], dt.float32, tag="ti")
                            nc.scalar.activation(tti[:, 0:cn], x_ps[:, 0:cn],
                                                 AF.Identity,
                                                 bias=mag_bias[:, 0:1], scale=1.0)
                            nc.scalar.activation(r[:, 0:cn], tti[:, 0:cn],
                                                 AF.Identity,
                                                 bias=nmag_bias[:, 0:1], scale=1.0)
                        else:
                            nc.vector.tensor_scalar(r[:, 0:cn], x_ps[:, 0:cn],
                                                    MAGIC, MAGIC,
                                                    ALU.add, ALU.subtract)
                        for qo in range(0, cn, 512):
                            qn = min(512, cn - qo)
                            nc.tensor.matmul(
                                x_ps[:, qo:qo + qn], negi[:], r[:, qo:qo + qn],
                                start=False, stop=True)
                        s = spool.tile([P, xchunk], dt.bfloat16, tag="s")
                        nc.scalar.activation(s[:, 0:cn], x_ps[:, 0:cn], AF.Sin,
                                             bias=zero_bias[:, 0:1],
                                             scale=2.0 * PI)
                        for qo in range(0, cn, 512):
                            qn = min(512, cn - qo)
                            q = (c0 + qo - s0) // 512
                            go = (c0 + qo - s0) % 512
                            nc.tensor.matmul(
                                hw_g[:, q, go:go + qn],
                                wsel[:, kg, :], s[:, qo:qo + qn],
                                start=(kg == 0), stop=(kg == 29))

                # audio = loud*(noise + mix*(hw - noise)) for this section
                hw_flat = hw_g[:].rearrange("p q f -> p (q f)")[:, 0:sn_]
                sl = slice(s0, s0 + sn_)
                e1 = pool.tile([64, FD], dt.float32, tag="escr", bufs=3, name=f"e1_{g}")
                nc.vector.tensor_tensor(e1[:, sl], hw_flat,
                                        noises[g][:, sl], ALU.subtract)
                epi_eng = nc.gpsimd if pool_epi else nc.vector
                e2 = pool.tile([64, FD], dt.float32, tag="escr", bufs=3, name=f"e2_{g}")
                epi_eng.tensor_tensor(e2[:, sl], e1[:, sl],
                                      mixs[g][:, sl], ALU.mult)
                e3 = pool.tile([64, FD], dt.float32, tag="escr", bufs=3, name=f"e3_{g}")
                epi_eng.tensor_tensor(e3[:, sl], e2[:, sl],
                                      noises[g][:, sl], ALU.add)
                epi_eng.tensor_tensor(audio[:, sl], e3[:, sl],
                                      louds[g][:, sl], ALU.mult)

        for g in range(2):
            audio = audios[g]
            # per-row peak: free-dim abs-max, then 32x32 transpose trick
            pk = pool.tile([64, 1], dt.float32, tag="pk")
            nc.vector.tensor_reduce(pk[:], audio[:], axis=mybir.AxisListType.X,
                                    op=ALU.max, apply_absolute_value=True)
            pkr = pool.tile([64, 32], dt.float32, tag="pkr")
            nc.vector.tensor_copy(pkr[:], pk[:, 0:1].to_broadcast((64, 32)))
            pkt = pool.tile([64, 32], dt.float32, tag="pkt")
            nc.vector.transpose(pkt[:], pkr[:])
            rowmax = pool.tile([64, 1], dt.float32, tag="rowmax")
            nc.vector.tensor_reduce(rowmax[:], pkt[:],
                                    axis=mybir.AxisListType.X, op=ALU.max)
            pke = pool.tile([64, 1], dt.float32, tag="pke")
            nc.vector.tensor_scalar(pke[:], rowmax[:], 1e-6, None, ALU.add)
            rcp = pool.tile([64, 1], dt.float32, tag="rcp")
            nc.vector.reciprocal(rcp[:], pke[:])
            outt = pool.tile([64, FD], dt.float32, tag="escr", bufs=3, name="outt")
            nc.vector.tensor_scalar(outt[:], audio[:], rcp[:, 0:1],
                                    None, ALU.mult)
            nc.sync.dma_start(out_d.ap()[64 * g:64 * g + 64, :], outt[:])

        if rep_ctx is not None:
            rep_ctx.__exit__(None, None, None)

    nc.compile()
    return nc


def kernel(f0, loudness, harmonic_mix, noise):
    if "nc" not in _cache:
        _cache["nc"] = _build()
        _cache["consts"] = _consts()
    nc = _cache["nc"]
    consts = _cache["consts"]

    def shard(a, c):
        return np.ascontiguousarray(
            a[c * RPC:(c + 1) * RPC].astype(f32, copy=False).reshape(P, FD))

    in_maps = []
    for c in range(NCORES):
        in_maps.append({
            "f0": shard(f0, c),
            "loud": shard(loudness, c),
            "mix": shard(harmonic_mix, c),
            "noise": shard(noise, c),
            **consts,
        })

    res = bass_utils.run_bass_kernel_spmd(nc, in_maps, core_ids=list(range(NCORES)))
    outs = [res.results[c]["audio"].reshape(RPC, T) for c in range(NCORES)]
    return np.concatenate(outs, axis=0)



# revision 2
# speedup vs baseline: 1328.6047x; 1328.6047x over previous
"""DDSP core synthesizer kernel for Trainium2 (8 NeuronCores, data-parallel).

Reference computation (per row of B=32, T=64000):
    f0_hz = 20*exp(f0); phase = cumsum(2*pi*f0_hz/SR)
    hw    = sum_k sin(phase*k)/k   (k = 1..60)
    audio = mix*hw*loud + (1-mix)*noise*loud;  out = audio / (max|audio| + 1e-6)

Device algorithm (everything in "turns" = phase/2pi), natural layout
[128 partitions = 4 rows x 32 blocks, 2000 free = time-in-block]:
    inc  = exp(f0 + ln(20/SR))                    [ACT Exp]
    u    = blocked cumsum + triangular-matmul offs [DVE scan + PE]
    u1   = frac(u)  in [-0.5, 0.5]                 [custom DVE FRAC_AFFINE]
    per harmonic k = 1..60:
        v_k = frac(k*u1)                           [custom DVE FRAC_AFFINE, 1 op]
        s_k = sin(2pi*v_k) -> bf16                 [ACT Sin, scale=2pi]
        hw += diag(1/k) @ s_k                      [PE accumulate in PSUM]
    epilogue: audio = A*hw + B with A = loud*mix, B = loud*noise*(1-mix)
              (A, B precomputed on GpSimd during the k-loop);
              peak-normalize per row (free-reduce + 32x32 transpose trick).

The custom DVE op (v = x - ((x+M)-M), x = Src0*C0 + C1, M = magic rint
constant) replaces the baseline's two extra PE passes (f16 x-gen matmul +
negi-subtract matmul) and the PSUM-sourced DVE rint, cutting PE work per
pass from 3 matmul sweeps to 1 and DVE work to a single SBUF-sourced op.

Sharding: pure data parallel, 4 rows per core, SPMD on cores 0-7.
"""

import sys

sys.path.insert(0, "/opt/trn_rl_repo")

import numpy as np
import ml_dtypes
from contextlib import ExitStack

import concourse.bass as bass
import concourse.tile as tile
from concourse import bacc, mybir
from concourse import bass_utils

f32 = np.float32
dt = mybir.dt

SR = 44100.0
H = 60                      # harmonics
B, T = 32, 64000
NCORES = 8
RPC = B // NCORES           # rows per core = 4
P = 128                     # SBUF partitions
FD = T * RPC // P           # free dim of master tiles = 2000
BPR = P // RPC              # blocks per row = 32
PI = float(np.pi)
MAGIC = float(1.5 * 2.0 ** 23)
EXP_BIAS = float(np.log(20.0 / SR))

_cache = {}


def _register_frac_op():
    """Register FRAC_AFFINE_ANT: out = x - ((x + M) - M), x = Src0*C0 + C1.

    C0/C1 are per-partition scalars (or literals), M = imm2 (the fp32 magic
    rint constant). 5 ALU stages, well under the 8-stage DVE budget."""
    if "op" in _cache.get("frac", {}):
        return _cache["frac"]["op"]
    from concourse import dve_ops
    from concourse.dve_spec import Spec, Src0, C0, C1, C2, lower
    from concourse.dve_spec import _has_src1 as has_src1
    from concourse.dve_uop import DveOpSpec
    from concourse.dve_table_gen import dve_ver_for

    name = "FRAC_AFFINE_ANT"

    def ref(in0, in1, s0, s1, imm2):
        x = (in0.astype(f32) * f32(s0) + f32(s1)).astype(f32)
        t = (x + f32(imm2)).astype(f32)
        w = (t - f32(imm2)).astype(f32)
        return (x - w).astype(f32)

    x = Src0 * C0 + C1
    spec = Spec(body=x - ((x + C2) - C2), reference=ref)

    if name not in dve_ops._SUB_OPCODE_FOR_NAME:
        row = max(dve_ops._SUB_OPCODE_FOR_NAME.values()) + 1
        assert row < 0x20
        dve_ops._SUB_OPCODE_FOR_NAME[name] = row

    ver = dve_ver_for("TRN2")
    tmp = DveOpSpec(
        name=name,
        opcode=dve_ops.get_dve_sub_opcode(name),
        uops=lower(spec, ver=ver),
        rd1_en=has_src1(spec),
    )
    op = dve_ops.DveOp(name, spec, subdim=False, uops_sha={ver: tmp.sha(ver)})
    if not any(o.name == name for o in dve_ops.OPS):
        dve_ops.OPS.append(op)
    dve_ops.CUSTOM_DVE_SPECS[name] = spec
    _cache["frac"] = {"op": op}
    return op


def _consts():
    # lt: exclusive-prefix matmul weights. offs[m] = sum_k lt[k, m] * totals[k]
    kk, mm_ = np.meshgrid(np.arange(P), np.arange(P), indexing="ij")
    lt = ((kk // BPR == mm_ // BPR) & (kk % BPR < mm_ % BPR)).astype(f32)

    # diags[k] = diag(1/(k+1)) as 128x128 lhsT for the PSUM accumulate
    diags = np.zeros((H, P, P), dtype=np.float64)
    for k in range(H):
        np.fill_diagonal(diags[k], 1.0 / (k + 1))
    diags = diags.astype(ml_dtypes.bfloat16)
    return {"lt": lt, "diags": diags}


def _build():
    frac_op = _register_frac_op()
    nc = bacc.Bacc("TRN2", target_bir_lowering=False, debug=False,
                   enable_asserts=True, num_devices=NCORES)

    f0_d = nc.dram_tensor("f0", [P, FD], dt.float32, kind="ExternalInput")
    loud_d = nc.dram_tensor("loud", [P, FD], dt.float32, kind="ExternalInput")
    mix_d = nc.dram_tensor("mix", [P, FD], dt.float32, kind="ExternalInput")
    noise_d = nc.dram_tensor("noise", [P, FD], dt.float32, kind="ExternalInput")
    lt_d = nc.dram_tensor("lt", [P, P], dt.float32, kind="ExternalInput")
    diags_d = nc.dram_tensor("diags", [H, P, P], dt.bfloat16, kind="ExternalInput")
    out_d = nc.dram_tensor("audio", [P, FD], dt.float32, kind="ExternalOutput")

    AF = mybir.ActivationFunctionType
    ALU = mybir.AluOpType

    with tile.TileContext(nc) as tc, ExitStack() as ctx:
        pool = ctx.enter_context(tc.tile_pool(name="sb", bufs=1))
        vpool = ctx.enter_context(tc.tile_pool(name="vp", bufs=3))
        spool = ctx.enter_context(tc.tile_pool(name="sp", bufs=3))
        hpool = ctx.enter_context(tc.tile_pool(name="hps", bufs=1, space="PSUM"))
        opool = ctx.enter_context(tc.tile_pool(name="ops", bufs=1, space="PSUM"))

        exp_bias = pool.tile([P, 1], dt.float32, tag="cbias_exp")
        nc.vector.memset(exp_bias[:], EXP_BIAS)
        zero_bias = pool.tile([P, 1], dt.float32, tag="cbias_zero")
        nc.vector.memset(zero_bias[:], 0.0)

        # ---- input DMA ----
        f0 = pool.tile([P, FD], dt.float32, tag="scr", bufs=4, name="f0")
        nc.sync.dma_start(f0[:], f0_d.ap())
        lt = pool.tile([P, P], dt.float32)
        nc.gpsimd.dma_start(lt[:], lt_d.ap())
        diags = pool.tile([P, H, P], dt.bfloat16)
        nc.gpsimd.dma_start(diags[:], diags_d.ap().rearrange("k p m -> p k m"))
        loud = pool.tile([P, FD], dt.float32, tag="loud")
        nc.scalar.dma_start(loud[:], loud_d.ap())
        mix = pool.tile([P, FD], dt.float32, tag="mix")
        nc.scalar.dma_start(mix[:], mix_d.ap())
        noise = pool.tile([P, FD], dt.float32, tag="noise")
        nc.scalar.dma_start(noise[:], noise_d.ap())

        # ---- stage 1: phase accumulation (turns) ----
        inc = pool.tile([P, FD], dt.float32, tag="scr", bufs=4, name="inc")
        nc.scalar.activation(inc[:], f0[:], AF.Exp, bias=exp_bias[:, 0:1], scale=1.0)

        local = pool.tile([P, FD], dt.float32, tag="scr", bufs=4, name="local")
        nc.vector.tensor_tensor_scan(local[:], inc[:], inc[:], 0.0,
                                     ALU.add, ALU.bypass)

        offs_ps = opool.tile([P, 1], dt.float32, tag="offs")
        nc.tensor.matmul(offs_ps[:], lt[:], local[:, FD - 1:FD],
                         start=True, stop=True)
        offs = pool.tile([P, 1], dt.float32)
        nc.vector.tensor_copy(offs[:], offs_ps[:])

        # u1 = frac(local + offs) in one custom-DVE op
        u1 = pool.tile([P, FD], dt.float32, tag="u1")
        nc.vector._custom_dve(frac_op, out=u1[:], in0=local[:],
                              s0=1.0, s1=offs[:, 0:1], imm2=MAGIC)

        # ---- epilogue prework on GpSimd (overlaps the k-loop) ----
        # audio = A*hw + B;  A = loud*mix, B = loud*noise*(1-mix)
        A = pool.tile([P, FD], dt.float32, tag="A")
        nc.gpsimd.tensor_tensor(A[:], loud[:], mix[:], ALU.mult)
        ln_ = pool.tile([P, FD], dt.float32, tag="ln")
        nc.gpsimd.tensor_tensor(ln_[:], loud[:], noise[:], ALU.mult)
        lnm = pool.tile([P, FD], dt.float32, tag="lnm")
        nc.gpsimd.tensor_tensor(lnm[:], ln_[:], mix[:], ALU.mult)
        Bt = pool.tile([P, FD], dt.float32, tag="Bt")
        nc.gpsimd.tensor_tensor(Bt[:], ln_[:], lnm[:], ALU.subtract)

        # ---- k-loop: v_k = frac(k*u1); s_k = sin(2pi v_k); hw += s_k/k ----
        hw = hpool.tile([P, 4, 512], dt.float32, tag="hw")
        for k in range(1, H + 1):
            v = vpool.tile([P, FD], dt.float32, tag="v")
            nc.vector._custom_dve(frac_op, out=v[:], in0=u1[:],
                                  s0=float(k), s1=0.0, imm2=MAGIC)
            s = spool.tile([P, FD], dt.bfloat16, tag="s")
            nc.scalar.activation(s[:], v[:], AF.Sin,
                                 bias=zero_bias[:, 0:1], scale=2.0 * PI)
            for qo in range(0, FD, 512):
                qn = min(512, FD - qo)
                nc.tensor.matmul(hw[:, qo // 512, 0:qn],
                                 diags[:, k - 1, :], s[:, qo:qo + qn],
                                 start=(k == 1), stop=(k == H))

        # ---- epilogue: audio = A*hw + B, then per-row peak normalize ----
        hw_flat = hw[:].rearrange("p q f -> p (q f)")[:, 0:FD]
        e1 = pool.tile([P, FD], dt.float32, tag="e1")
        nc.vector.tensor_tensor(e1[:], A[:], hw_flat, ALU.mult)
        audio = pool.tile([P, FD], dt.float32, tag="audio")
        nc.vector.tensor_tensor(audio[:], e1[:], Bt[:], ALU.add)

        # per-row peak: free-dim abs-max then 32x32 block transpose trick
        pk = pool.tile([P, 1], dt.float32, tag="pk")
        nc.vector.tensor_reduce(pk[:], audio[:], axis=mybir.AxisListType.X,
                                op=ALU.max, apply_absolute_value=True)
        pkr = pool.tile([P, 32], dt.float32, tag="pkr")
        nc.vector.tensor_copy(pkr[:], pk[:, 0:1].to_broadcast((P, 32)))
        pkt = pool.tile([P, 32], dt.float32, tag="pkt")
        nc.vector.transpose(pkt[:], pkr[:])
        rowmax = pool.tile([P, 1], dt.float32, tag="rowmax")
        nc.vector.tensor_reduce(rowmax[:], pkt[:],
                                axis=mybir.AxisListType.X, op=ALU.max)
        pke = pool.tile([P, 1], dt.float32, tag="pke")
        nc.vector.tensor_scalar(pke[:], rowmax[:], 1e-6, None, ALU.add)
        rcp = pool.tile([P, 1], dt.float32, tag="rcp")
        nc.vector.reciprocal(rcp[:], pke[:])
        outt = pool.tile([P, FD], dt.float32, tag="outt")
        nc.vector.tensor_scalar(outt[:], audio[:], rcp[:, 0:1], None, ALU.mult)
        nc.sync.dma_start(out_d.ap(), outt[:])

    nc.compile()
    return nc


def kernel(f0, loudness, harmonic_mix, noise):
    if "nc" not in _cache:
        _cache["nc"] = _build()
        _cache["consts"] = _consts()
    nc = _cache["nc"]
    consts = _cache["consts"]

    def shard(a, c):
        return np.ascontiguousarray(
            a[c * RPC:(c + 1) * RPC].astype(f32, copy=False).reshape(P, FD))

    in_maps = []
    for c in range(NCORES):
        in_maps.append({
            "f0": shard(f0, c),
            "loud": shard(loudness, c),
            "mix": shard(harmonic_mix, c),
            "noise": shard(noise, c),
            **consts,
        })

    res = bass_utils.run_bass_kernel_spmd(nc, in_maps, core_ids=list(range(NCORES)))
    outs = [res.results[c]["audio"].reshape(RPC, T) for c in range(NCORES)]
    return np.concatenate(outs, axis=0)
